# revision 8
# baseline (speedup 1.0000x reference)
"""Trainium2 Bass kernel: MultiHeadCrossAttentionWithBias.

Reference computation (per batch b):
  q_u = scale*(u_enc @ wq + wq_b); k/v from e_enc (and vice versa)
  ue_w = softmax(q_u k_e^T + bpp + mask*-inf); u_ctx = ue_w @ v_e
  u_update = u_ctx @ wo + wo_b                     (same mirrored for e)

Sharding: the problem decomposes into 8 fully independent attention units:
(batch b, direction d) for b in 0..3, d in {u->e, e->u}. Core i = (d, b)
handles one unit end-to-end; no collectives needed.

Host prep is layout/precision only (transposes, slices, fp32->bf16
rounding of matmul operands); all FLOPs run on device.

Per-core inputs:
  encQT  [D=512, L=1024] bf16  query-side encoder, transposed
  encKT  [D=512, L=1024] bf16  key-side encoder, transposed
  bpp    [L, L] bf16           logit bias oriented [k, q]
  mask   [L, L] uint8          mask oriented [k, q]
  wq/wk/wv [D, 512] bf16, wo [512, D] bf16, biases f32

On-device math (per core), all matmul operands bf16 (FWL-friendly, no
fp32 slow paths; PSUM accumulation stays f32):
  qT[f, s] = scale*(wq^T encQT + wq_b)   (f = h*64+hd on partitions)
  kT[f, s] =        wk^T encKT + wk_b
  v[s, f]  =        encKT^T wv + wv_b    (+ fused ones column per head)
  CB[k, q] = bpp_w*bpp + bpp_b + (mask-1)*1e30   (DVE+gpsimd, -> bf16)
  per head h, k-chunk kc:
      S^T = CB[kc]                 (PE: identity-stationary matmul, start)
      S^T += kT_h^T qT_h           (PE accumulation, stop)
      E = exp(S^T)                 (ACT; no max-subtraction: logits O(10))
      [ctx^T; den] += [v_h | 1]^T E  (PE, PSUM accumulation over kc)
  The CB injection rides the PE (instead of a DVE add on the critical
  path) so the tensor engine stream stays dense: PE-HAM then holds the
  2.4 GHz clock state through the attention phase.
  rcp = approx_reciprocal(den); partition-broadcast via PE matmul with a
  2-row selector stationary (no DRAM bounce)
  ctxn[pair] = ctx^T * rcp  (DVE, odd head written to partitions 64..127)
  out[s, e] = sum_pair ctxn_p^T wo_p + wo_b   (PE + DVE bias-add eviction)
"""

import numpy as np
from contextlib import ExitStack

import ml_dtypes

import concourse.bass as bass
import concourse.tile as tile
import concourse.bacc as bacc
import concourse.mybir as mybir
from concourse.masks import make_identity
from concourse import bass_utils

F32 = mybir.dt.float32
U8 = mybir.dt.uint8
BF16 = mybir.dt.bfloat16
AF = mybir.ActivationFunctionType
ALU = mybir.AluOpType

B, L, D, H, HD = 4, 1024, 512, 8, 64
P = 128
FH = H * HD            # 512
SCALE = 1.0 / np.sqrt(HD)
NEG = -1.0e30
N_CORES = 8


def bcast_ap(dram_ap, parts):
    """Partition-step-0 broadcast AP over a DRAM row."""
    return bass.AP(tensor=dram_ap.tensor, offset=dram_ap.offset,
                   ap=[[0, parts]] + list(dram_ap.ap))


def build_module():
    nc = bacc.Bacc("TRN2", target_bir_lowering=False, debug=False)

    encQT_d = nc.dram_tensor("encQT", [D, L], BF16, kind="ExternalInput")
    encKT_d = nc.dram_tensor("encKT", [D, L], BF16, kind="ExternalInput")
    wq_d = nc.dram_tensor("wq", [D, FH], BF16, kind="ExternalInput")
    wk_d = nc.dram_tensor("wk", [D, FH], BF16, kind="ExternalInput")
    wv_d = nc.dram_tensor("wv", [D, FH], BF16, kind="ExternalInput")
    wo_d = nc.dram_tensor("wo", [FH, D], BF16, kind="ExternalInput")
    bpp_d = nc.dram_tensor("bpp", [L, L], BF16, kind="ExternalInput")
    mask_d = nc.dram_tensor("mask", [L, L], U8, kind="ExternalInput")
    wqb_d = nc.dram_tensor("wqb", [FH], F32, kind="ExternalInput")
    wkb_d = nc.dram_tensor("wkb", [FH], F32, kind="ExternalInput")
    wvb_d = nc.dram_tensor("wvb", [FH], F32, kind="ExternalInput")
    wob_d = nc.dram_tensor("wob", [D], F32, kind="ExternalInput")
    bppw_d = nc.dram_tensor("bppw", [1, 1], F32, kind="ExternalInput")
    bppb_d = nc.dram_tensor("bppb", [1, 1], F32, kind="ExternalInput")
    out_d = nc.dram_tensor("out", [L, D], F32, kind="ExternalOutput")

    with tile.TileContext(nc) as tc, ExitStack() as ctx:
        const = ctx.enter_context(tc.tile_pool(name="const", bufs=1))
        qkT_p = ctx.enter_context(tc.tile_pool(name="qkT", bufs=8))
        v_p = ctx.enter_context(tc.tile_pool(name="v", bufs=8))
        wo_p = ctx.enter_context(tc.tile_pool(name="wo", bufs=4))
        cb_p = ctx.enter_context(tc.tile_pool(name="cb", bufs=8))
        ps_s = tc.alloc_tile_pool(name="ps_s", bufs=2, space="PSUM")
        ps_c = tc.alloc_tile_pool(name="ps_c", bufs=2, space="PSUM")
        ps_r = tc.alloc_tile_pool(name="ps_r", bufs=2, space="PSUM")

        # ---- small bias prep (tiny DMAs) ----
        # bpp_w / bpp_b broadcast to [128,1] columns
        bw_col = const.tile([P, 1], F32)
        nc.gpsimd.dma_start(bw_col[:], bcast_ap(bppw_d.ap()[0:1, :], P))
        bb_col = const.tile([P, 1], F32)
        nc.gpsimd.dma_start(bb_col[:], bcast_ap(bppb_d.ap()[0:1, :], P))
        # projection biases
        wqb_raw = const.tile([P, 4], F32)
        nc.gpsimd.dma_start(wqb_raw[:], wqb_d.ap().rearrange("(c p) -> p c", p=P))
        wqb_sc = const.tile([P, 4], F32)
        nc.vector.tensor_scalar_mul(wqb_sc[:], wqb_raw[:], float(SCALE))
        wkb_c = const.tile([P, 4], F32)
        nc.gpsimd.dma_start(wkb_c[:], wkb_d.ap().rearrange("(c p) -> p c", p=P))
        wvb_bc = const.tile([P, FH], F32)
        nc.gpsimd.dma_start(wvb_bc[:], bcast_ap(wvb_d.ap(), P))
        wob_bc = const.tile([P, D], F32)
        nc.gpsimd.dma_start(wob_bc[:], bcast_ap(wob_d.ap(), P))

        # identity stationary for the CB->PSUM injection matmuls
        ident = const.tile([P, P], BF16)
        make_identity(nc, ident[:])
        # selector for the reciprocal partition-broadcast (engine writes
        # must start at partition 0/32/64/96, so the two live rows sit at
        # partitions 0 and 32): sel[0, 0:64] = 1, sel[32, 64:128] = 1
        sel = const.tile([33, P], BF16)
        nc.gpsimd.memset(sel[:], 0.0)
        nc.gpsimd.memset(sel[0:1, 0:HD], 1.0)
        nc.gpsimd.memset(sel[32:33, HD:P], 1.0)

        # ---- projections ----
        cb = []
        qT, kT, v_aug = [], [], []
        with tc.tile_pool(name="enc", bufs=8) as enc_p, \
             tc.tile_pool(name="wqkv", bufs=12) as w_p:
            eq, ek = [], []
            wq_t, wk_t, wv_t = [], [], []
            # load in first-use order: wq -> encQT -> wk -> encKT -> wv
            for w_dram, wlst, elst, edram in (
                (wq_d, wq_t, eq, encQT_d), (wk_d, wk_t, ek, encKT_d),
                (wv_d, wv_t, None, None),
            ):
                for dc in range(4):
                    t = w_p.tile([P, FH], BF16, tag="w",
                                 name=f"w_{w_dram.name}{dc}")
                    nc.sync.dma_start(t[:], w_dram.ap()[dc * P:(dc + 1) * P, :])
                    wlst.append(t)
                if elst is None:
                    continue
                for dc in range(4):
                    t = enc_p.tile([P, L], BF16, tag="enc",
                                   name=f"enc_{edram.name}{dc}")
                    nc.sync.dma_start(t[:], edram.ap()[dc * P:(dc + 1) * P, :])
                    elst.append(t)

            # ---- combined bias CB[k, q] = (bpp*w + b) + (mask-1)*1e30 ----
            # emitted after projection-critical loads so its DMA traffic
            # does not delay the first matmuls; compute overlaps
            # projections. (m*1e30) + (-1e30) is exact for m in {0,1}.
            negbig = const.tile([P, 1], F32)
            nc.vector.memset(negbig[:], NEG)
            cbt_p = tc.alloc_tile_pool(name="cbtmp", bufs=3)
            for kc in range(8):
                m_t = cbt_p.tile([P, L], U8, tag="m", name=f"m{kc}")
                nc.sync.dma_start(m_t[:], mask_d.ap()[kc * P:(kc + 1) * P, :])
                mn_t = cbt_p.tile([P, L], F32, tag="mn", name=f"mn{kc}")
                nc.scalar.activation(mn_t[:], m_t[:], AF.Identity,
                                     bias=negbig[:], scale=-NEG)
                b_t = cbt_p.tile([P, L], BF16, tag="b", name=f"b{kc}")
                nc.sync.dma_start(b_t[:], bpp_d.ap()[kc * P:(kc + 1) * P, :])
                bs_t = cbt_p.tile([P, L], F32, tag="bs", name=f"bs{kc}")
                nc.vector.tensor_scalar(bs_t[:], b_t[:], bw_col[:, 0:1],
                                        bb_col[:, 0:1], ALU.mult, ALU.add)
                c_t = cb_p.tile([P, L], BF16, tag="cb", name=f"cb{kc}")
                nc.gpsimd.tensor_add(c_t[:], mn_t[:], bs_t[:])
                cb.append(c_t)
            cbt_p.release()

            # qT / kT: [f, s] packed two heads per 128-partition chunk
            for which, w_t, enc_t, out_list in (
                ("q", wq_t, eq, qT), ("k", wk_t, ek, kT),
            ):
                for pc in range(4):
                    o = qkT_p.tile([P, L], BF16, tag="qkT",
                                   name=f"{which}T{pc}")
                    for sh in range(2):
                        ps = ps_s.tile([P, 512], F32, tag="ps_s",
                                       name=f"ps_{which}{pc}_{sh}")
                        for dc in range(4):
                            nc.tensor.matmul(
                                ps[:],
                                w_t[dc][:, pc * P:(pc + 1) * P],
                                enc_t[dc][:, sh * 512:(sh + 1) * 512],
                                start=(dc == 0), stop=(dc == 3))
                        sl = slice(sh * 512, (sh + 1) * 512)
                        if which == "q":
                            nc.scalar.activation(o[:, sl], ps[:], AF.Identity,
                                                 bias=wqb_sc[:, pc:pc + 1],
                                                 scale=float(SCALE))
                        else:
                            nc.scalar.activation(o[:, sl], ps[:], AF.Identity,
                                                 bias=wkb_c[:, pc:pc + 1],
                                                 scale=1.0)
                    out_list.append(o)

            # v: [s, f] with ones column interleaved per head ([128, 8*65])
            for sc in range(8):
                ps = ps_s.tile([P, 512], F32, tag="ps_s", name=f"ps_v{sc}")
                for dc in range(4):
                    nc.tensor.matmul(ps[:], ek[dc][:, sc * P:(sc + 1) * P],
                                     wv_t[dc][:], start=(dc == 0),
                                     stop=(dc == 3))
                va = v_p.tile([P, H * (HD + 1)], BF16, tag="v", name=f"v{sc}")
                vg = va[:].rearrange("p (h c) -> p h c", c=HD + 1)
                nc.vector.scalar_tensor_tensor(
                    vg[:, :, 0:HD],
                    ps[:].rearrange("p (h c) -> p h c", c=HD), 1.0,
                    wvb_bc[:].rearrange("p (h c) -> p h c", c=HD),
                    ALU.bypass, ALU.add)
                nc.vector.memset(vg[:, :, HD:HD + 1], 1.0)
                v_aug.append(va)

        # ---- wo loads (late: not projection-critical) ----
        wo_t = []
        for p_ in range(4):
            t = wo_p.tile([P, D], BF16, tag="wo", name=f"wo{p_}")
            nc.sync.dma_start(t[:], wo_d.ap()[p_ * P:(p_ + 1) * P, :])
            wo_t.append(t)

        # ---- attention ----
        # Per (head, kc): CB inject (PE, identity matmul, start) -> QK
        # accumulate (PE, stop) -> exp (ACT, -> bf16) -> PV (PE, bf16).
        # PV is emitted with a lag so the in-order PE stream never waits
        # on exp; everything between exp results is pure PE work.
        ctxn_p = ctx.enter_context(tc.tile_pool(name="ctxn", bufs=4))
        ctxr_p = ctx.enter_context(tc.tile_pool(name="ctxr", bufs=2))
        den_p = ctx.enter_context(tc.tile_pool(name="den", bufs=4))
        ctxn = [None] * 4
        with tc.tile_pool(name="e", bufs=8) as e_p:
            for h in range(H):
                LAG = 4 if h == 0 else 3
                o = (h % 2) * HD
                pc = h // 2
                c_ps = [ps_c.tile([HD + 1, 512], F32, tag="ps_c",
                                  name=f"c_ps_{h}_{i}")
                        for i in range(2)]
                e_ts = {}
                for kc in range(8 + LAG):
                    if kc < 8:
                        s_ps = ps_s.tile([P, L], F32, tag="ps_s",
                                         name=f"s_ps_{h}_{kc}")
                        for qh in range(2):
                            sl = slice(qh * 512, (qh + 1) * 512)
                            nc.tensor.matmul(
                                s_ps[:, sl], ident[:], cb[kc][:, sl],
                                start=True, stop=False)
                            nc.tensor.matmul(
                                s_ps[:, sl],
                                kT[pc][o:o + HD, kc * P:(kc + 1) * P],
                                qT[pc][o:o + HD, sl],
                                start=False, stop=True)
                        et = e_p.tile([P, L], BF16, tag="e",
                                      name=f"e_{h}_{kc}")
                        nc.scalar.activation(et[:], s_ps[:], AF.Exp)
                        e_ts[kc] = et
                    if kc >= LAG:
                        kp = kc - LAG
                        for qh in range(2):
                            sl = slice(qh * 512, (qh + 1) * 512)
                            nc.tensor.matmul(
                                c_ps[qh][:],
                                v_aug[kp][:, h * (HD + 1):(h + 1) * (HD + 1)],
                                e_ts[kp][:, sl],
                                start=(kp == 0), stop=(kp == 7))
                # evict raw ctx + denominator rows (ACT, partition-shifted),
                # freeing PSUM; normalize later from SBUF.
                if h % 2 == 0:
                    ctxn[pc] = ctxn_p.tile([P, L], BF16, tag="ctxn",
                                           name=f"ctxn{pc}")
                    ctxr = ctxr_p.tile([P, L], F32, tag="ctxr",
                                       name=f"ctxr{pc}")
                    den_sb = den_p.tile([33, L], F32, tag="den",
                                        name=f"den{pc}")
                    ctxr_hold = (ctxr, den_sb)
                else:
                    ctxr, den_sb = ctxr_hold
                dr = (h % 2) * 32
                for qh in range(2):
                    sl = slice(qh * 512, (qh + 1) * 512)
                    nc.scalar.copy(ctxr[o:o + HD, sl], c_ps[qh][0:HD, :])
                    nc.scalar.copy(den_sb[dr:dr + 1, sl],
                                   c_ps[qh][HD:HD + 1, :])
                if h % 2 == 1:
                    # reciprocal of the pair's denominators (rows 0 / 32),
                    # then partition-broadcast via PE: rows 0..63 get
                    # rcp[0], rows 64..127 get rcp[32].
                    rcp = den_p.tile([33, L], BF16, tag="rcp",
                                     name=f"rcp{pc}")
                    nc.vector.memset(rcp[:], 0.0)
                    with nc.allow_low_precision(
                            reason="rcp feeds a bf16 matmul broadcast"):
                        nc.vector.reciprocal(rcp[0:1, :], den_sb[0:1, :])
                        nc.vector.reciprocal(rcp[32:33, :], den_sb[32:33, :])
                    for qh in range(2):
                        sl = slice(qh * 512, (qh + 1) * 512)
                        rb_ps = ps_r.tile([P, 512], F32, tag="ps_r",
                                          name=f"rb_{pc}_{qh}")
                        nc.tensor.matmul(rb_ps[:], sel[:], rcp[:, sl],
                                         start=True, stop=True)
                        nc.vector.scalar_tensor_tensor(
                            ctxn[pc][:, sl], ctxr[:, sl], 1.0, rb_ps[:],
                            ALU.bypass, ALU.mult)

        # ---- output projection ----
        # p-major emission: all pair-0 matmuls first, so the PE only waits
        # on the last pair's normalize chain for the final 8 matmuls.
        ps_r.release()
        ps_c.release()
        ps_s.release()
        ps_o = tc.alloc_tile_pool(name="ps_o", bufs=8, space="PSUM")
        with tc.tile_pool(name="outp", bufs=3) as out_p:
            o_ps = [ps_o.tile([P, D], F32, tag="ps_o", name=f"o_ps{st}")
                    for st in range(8)]
            for p_ in range(4):
                for st in range(8):
                    nc.tensor.matmul(o_ps[st][:],
                                     ctxn[p_][:, st * P:(st + 1) * P],
                                     wo_t[p_][:],
                                     start=(p_ == 0), stop=(p_ == 3))
            for st in range(8):
                o_t = out_p.tile([P, D], F32, tag="out", name=f"out{st}")
                nc.vector.scalar_tensor_tensor(
                    o_t[:], o_ps[st][:], 1.0, wob_bc[:], ALU.bypass, ALU.add)
                nc.sync.dma_start(out_d.ap()[st * P:(st + 1) * P, :], o_t[:])
        ps_o.release()

    nc.compile()
    return nc


def to_bf16(x):
    return np.asarray(x, np.float32).astype(ml_dtypes.bfloat16)


def shard_inputs(u_enc, e_enc, logit_bpp, ue_mask, eu_mask,
                 wq_k, wq_b, wk_k, wk_b, wv_k, wv_b, wo_k, wo_b,
                 bpp_w, bpp_b):
    """Build the 8 per-core input maps (layout + bf16 rounding only)."""
    u_enc = np.asarray(u_enc, np.float32)
    e_enc = np.asarray(e_enc, np.float32)
    bpp = np.asarray(logit_bpp, np.float32)
    ue_m = np.asarray(ue_mask).astype(np.uint8)
    eu_m = np.asarray(eu_mask).astype(np.uint8)
    com = dict(
        wq=to_bf16(np.asarray(wq_k, np.float32).reshape(D, FH)),
        wk=to_bf16(np.asarray(wk_k, np.float32).reshape(D, FH)),
        wv=to_bf16(np.asarray(wv_k, np.float32).reshape(D, FH)),
        wo=to_bf16(np.asarray(wo_k, np.float32).reshape(FH, D)),
        wqb=np.asarray(wq_b, np.float32).reshape(FH).copy(),
        wkb=np.asarray(wk_b, np.float32).reshape(FH).copy(),
        wvb=np.asarray(wv_b, np.float32).reshape(FH).copy(),
        wob=np.asarray(wo_b, np.float32).reshape(D).copy(),
        bppw=np.asarray(bpp_w, np.float32).reshape(1, 1).copy(),
        bppb=np.asarray(bpp_b, np.float32).reshape(1, 1).copy(),
    )
    uT = [to_bf16(u_enc[b].T) for b in range(B)]
    eT = [to_bf16(e_enc[b].T) for b in range(B)]
    bpp_bf = to_bf16(bpp)
    bppT_bf = to_bf16(np.ascontiguousarray(bpp.T))
    in_maps = []
    for i in range(N_CORES):
        d, b = divmod(i, B)
        if d == 0:      # u queries, e keys -> u_update[b]
            m = dict(encQT=uT[b], encKT=eT[b], bpp=bppT_bf,
                     mask=np.ascontiguousarray(ue_m[b, 0].T))
        else:           # e queries, u keys -> e_update[b]
            m = dict(encQT=eT[b], encKT=uT[b], bpp=bpp_bf,
                     mask=np.ascontiguousarray(eu_m[b, 0].T))
        m.update(com)
        in_maps.append(m)
    return in_maps


_NC = None


def kernel(**inputs):
    global _NC
    if _NC is None:
        _NC = build_module()
    in_maps = shard_inputs(**inputs)
    res = bass_utils.run_bass_kernel_spmd(
        _NC, in_maps, core_ids=list(range(N_CORES)))
    u_update = np.stack([res.results[b]["out"] for b in range(B)])
    e_update = np.stack([res.results[B + b]["out"] for b in range(B)])
    return u_update, e_update


if __name__ == "__main__":
    # single-core CoreSim check of one (direction, batch) unit
    from concourse.bass_interp import CoreSim

    rng = np.random.default_rng(0)
    u = rng.standard_normal((B, L, D)).astype(np.float32)
    e = rng.standard_normal((B, L, D)).astype(np.float32)
    bpp = rng.standard_normal((L, L)).astype(np.float32)
    uem = (rng.random((B, 1, L, L)) < 0.9)
    eum = (rng.random((B, 1, L, L)) < 0.9)
    w = 1.0 / np.sqrt(D)
    wq = (rng.standard_normal((D, H, HD)) * w).astype(np.float32)
    wk = (rng.standard_normal((D, H, HD)) * w).astype(np.float32)
    wv = (rng.standard_normal((D, H, HD)) * w).astype(np.float32)
    wo = (rng.standard_normal((H, HD, D)) / np.sqrt(FH)).astype(np.float32)
    zq = (rng.standard_normal((H, HD)) * 0.1).astype(np.float32)
    zo = (rng.standard_normal((D,)) * 0.1).astype(np.float32)

    nc = build_module()
    in_maps = shard_inputs(u, e, bpp, uem, eum, wq, zq, wk, zq, wv, zq,
                           wo, zo, np.float32(1.3), np.float32(-0.2))

    core = 0
    sim = CoreSim(nc, trace=False)
    for k, vv in in_maps[core].items():
        sim.tensor(k)[:] = vv
    sim.simulate(check_with_hw=False)
    got = np.array(sim.tensor("out"))

    def ref_unit(encQ, encK, bias_qk, mask_qk):
        q = SCALE * (encQ @ wq.reshape(D, FH) + zq.reshape(FH))
        kk = encK @ wk.reshape(D, FH) + zq.reshape(FH)
        vv = encK @ wv.reshape(D, FH) + zq.reshape(FH)
        accum = np.zeros((L, D), np.float64)
        for h in range(H):
            qi = q[:, h * HD:(h + 1) * HD]
            ki = kk[:, h * HD:(h + 1) * HD]
            vi = vv[:, h * HD:(h + 1) * HD]
            s = qi @ ki.T + bias_qk
            s = np.where(mask_qk, s, -np.inf)
            s = s - s.max(-1, keepdims=True)
            p_ = np.exp(s)
            p_ /= p_.sum(-1, keepdims=True)
            accum += (p_ @ vi) @ wo[h]
        return (accum + zo).astype(np.float32)

    bq = 1.3 * bpp + -0.2
    exp_out = ref_unit(u[0], e[0], bq, uem[0, 0])
    err = np.abs(got - exp_out).max() / np.abs(exp_out).max()
    print("unit relerr vs numpy:", err)


# revision 15
# speedup vs baseline: 1.1283x; 1.1283x over previous
"""Trainium2 Bass kernel: MultiHeadCrossAttentionWithBias.

Reference computation (per batch b):
  q_u = scale*(u_enc @ wq + wq_b); k/v from e_enc (and vice versa)
  ue_w = softmax(q_u k_e^T + bpp + mask*-inf); u_ctx = ue_w @ v_e
  u_update = u_ctx @ wo + wo_b                     (same mirrored for e)

Sharding: the problem decomposes into 8 fully independent attention units:
(batch b, direction d) for b in 0..3, d in {u->e, e->u}. Core i = (d, b)
handles one unit end-to-end; no collectives needed.

Host prep is layout/precision only (transposes, slices, fp32->bf16
rounding of matmul operands); all FLOPs run on device.

Per-core inputs:
  encQT  [D=512, L=1024] bf16  query-side encoder, transposed
  encKT  [D=512, L=1024] bf16  key-side encoder, transposed
  bpp    [L, L] bf16           logit bias oriented [k, q]
  mask   [L, L] uint8          mask oriented [k, q]
  wq/wk/wv [D, 512] bf16, wo [512, D] bf16, biases f32

On-device math (per core), all matmul operands bf16 (FWL-friendly, no
fp32 slow paths; PSUM accumulation stays f32):
  qT[f, s] = scale*(wq^T encQT + wq_b)   (f = h*64+hd on partitions)
  kT[f, s] =        wk^T encKT + wk_b
  v[s, f]  =        encKT^T wv + wv_b    (+ fused ones column per head)
  CB[k, q] = bpp_w*bpp + bpp_b + (mask-1)*1e30   (DVE+gpsimd, -> bf16)
  per head h, k-chunk kc:
      S^T = CB[kc]                 (PE: identity-stationary matmul, start)
      S^T += kT_h^T qT_h           (PE accumulation, stop)
      E = exp(S^T)                 (ACT; no max-subtraction: logits O(10))
      [ctx^T; den] += [v_h | 1]^T E  (PE, PSUM accumulation over kc)
  The CB injection rides the PE (instead of a DVE add on the critical
  path) so the tensor engine stream stays dense: PE-HAM then holds the
  2.4 GHz clock state through the attention phase.
  rcp = approx_reciprocal(den); partition-broadcast via PE matmul with a
  2-row selector stationary (no DRAM bounce)
  ctxn[pair] = ctx^T * rcp  (DVE, odd head written to partitions 64..127)
  out[s, e] = sum_pair ctxn_p^T wo_p + wo_b   (PE + DVE bias-add eviction)
"""

import numpy as np
from contextlib import ExitStack

import ml_dtypes

import concourse.bass as bass
import concourse.tile as tile
import concourse.bacc as bacc
import concourse.mybir as mybir
from concourse.masks import make_identity
from concourse import bass_utils

F32 = mybir.dt.float32
U8 = mybir.dt.uint8
BF16 = mybir.dt.bfloat16
AF = mybir.ActivationFunctionType
ALU = mybir.AluOpType

B, L, D, H, HD = 4, 1024, 512, 8, 64
P = 128
FH = H * HD            # 512
SCALE = 1.0 / np.sqrt(HD)
NEG = -1.0e30
N_CORES = 8


def bcast_ap(dram_ap, parts):
    """Partition-step-0 broadcast AP over a DRAM row."""
    return bass.AP(tensor=dram_ap.tensor, offset=dram_ap.offset,
                   ap=[[0, parts]] + list(dram_ap.ap))


def build_module():
    nc = bacc.Bacc("TRN2", target_bir_lowering=False, debug=False)

    encQT_d = nc.dram_tensor("encQT", [D, L], BF16, kind="ExternalInput")
    encKT_d = nc.dram_tensor("encKT", [D, L], BF16, kind="ExternalInput")
    wq_d = nc.dram_tensor("wq", [D, FH], BF16, kind="ExternalInput")
    wk_d = nc.dram_tensor("wk", [D, FH], BF16, kind="ExternalInput")
    wv_d = nc.dram_tensor("wv", [D, FH], BF16, kind="ExternalInput")
    wo_d = nc.dram_tensor("wo", [FH, D], BF16, kind="ExternalInput")
    bpp_d = nc.dram_tensor("bpp", [L, L], BF16, kind="ExternalInput")
    mask_d = nc.dram_tensor("mask", [L, L], U8, kind="ExternalInput")
    wqb_d = nc.dram_tensor("wqb", [FH], F32, kind="ExternalInput")
    wkb_d = nc.dram_tensor("wkb", [FH], F32, kind="ExternalInput")
    wvb_d = nc.dram_tensor("wvb", [FH], F32, kind="ExternalInput")
    wob_d = nc.dram_tensor("wob", [D], F32, kind="ExternalInput")
    bppw_d = nc.dram_tensor("bppw", [1, 1], F32, kind="ExternalInput")
    bppb_d = nc.dram_tensor("bppb", [1, 1], F32, kind="ExternalInput")
    out_d = nc.dram_tensor("out", [L, D], F32, kind="ExternalOutput")

    with tile.TileContext(nc) as tc, ExitStack() as ctx:
        const = ctx.enter_context(tc.tile_pool(name="const", bufs=1))
        qkT_p = ctx.enter_context(tc.tile_pool(name="qkT", bufs=8))
        v_p = ctx.enter_context(tc.tile_pool(name="v", bufs=8))
        wo_p = ctx.enter_context(tc.tile_pool(name="wo", bufs=4))
        cb_p = ctx.enter_context(tc.tile_pool(name="cb", bufs=8))
        ps_s = tc.alloc_tile_pool(name="ps_s", bufs=2, space="PSUM")
        ps_c = tc.alloc_tile_pool(name="ps_c", bufs=2, space="PSUM")
        ps_r = tc.alloc_tile_pool(name="ps_r", bufs=2, space="PSUM")

        # ---- small bias prep (tiny DMAs) ----
        # bpp_w / bpp_b broadcast to [128,1] columns
        bw_col = const.tile([P, 1], F32)
        nc.gpsimd.dma_start(bw_col[:], bcast_ap(bppw_d.ap()[0:1, :], P))
        bb_col = const.tile([P, 1], F32)
        nc.gpsimd.dma_start(bb_col[:], bcast_ap(bppb_d.ap()[0:1, :], P))
        # projection biases
        wqb_raw = const.tile([P, 4], F32)
        nc.gpsimd.dma_start(wqb_raw[:], wqb_d.ap().rearrange("(c p) -> p c", p=P))
        wqb_sc = const.tile([P, 4], F32)
        nc.vector.tensor_scalar_mul(wqb_sc[:], wqb_raw[:], float(SCALE))
        wkb_c = const.tile([P, 4], F32)
        nc.gpsimd.dma_start(wkb_c[:], wkb_d.ap().rearrange("(c p) -> p c", p=P))
        wvb_bc = const.tile([P, FH], F32)
        nc.gpsimd.dma_start(wvb_bc[:], bcast_ap(wvb_d.ap(), P))
        wob_bc = const.tile([P, D], F32)
        nc.gpsimd.dma_start(wob_bc[:], bcast_ap(wob_d.ap(), P))

        # identity stationary for the CB->PSUM injection matmuls
        ident = const.tile([P, P], BF16)
        make_identity(nc, ident[:])
        # selector for the denominator partition-broadcast (engine writes
        # must start at partition 0/32/64/96, so the two live rows sit at
        # partitions 0 and 32): sel[0, 0:64] = 1, sel[32, 64:128] = 1.
        # f32r keeps the denominator at full precision through the PE.
        F32R = mybir.dt.float32r
        sel = const.tile([33, P], F32R)
        nc.gpsimd.memset(sel[:].bitcast(F32), 0.0)
        nc.gpsimd.memset(sel[0:1, 0:HD].bitcast(F32), 1.0)
        nc.gpsimd.memset(sel[32:33, HD:P].bitcast(F32), 1.0)

        # ---- projections ----
        cb = []
        qT, kT, v_aug = [], [], []
        with tc.tile_pool(name="enc", bufs=8) as enc_p, \
             tc.tile_pool(name="wqkv", bufs=12) as w_p:
            eq, ek = [], []
            wq_t, wk_t, wv_t = [], [], []
            # load in first-use order: wq -> encQT -> wk -> encKT -> wv
            for w_dram, wlst, elst, edram in (
                (wq_d, wq_t, eq, encQT_d), (wk_d, wk_t, ek, encKT_d),
                (wv_d, wv_t, None, None),
            ):
                for dc in range(4):
                    t = w_p.tile([P, FH], BF16, tag="w",
                                 name=f"w_{w_dram.name}{dc}")
                    nc.sync.dma_start(t[:], w_dram.ap()[dc * P:(dc + 1) * P, :])
                    wlst.append(t)
                if elst is None:
                    continue
                for dc in range(4):
                    t = enc_p.tile([P, L], BF16, tag="enc",
                                   name=f"enc_{edram.name}{dc}")
                    nc.sync.dma_start(t[:], edram.ap()[dc * P:(dc + 1) * P, :])
                    elst.append(t)

            # ---- combined bias CB[k, q] = (bpp*w + b) + (mask-1)*1e30 ----
            # emitted after projection-critical loads so its DMA traffic
            # does not delay the first matmuls; compute overlaps
            # projections. (m*1e30) + (-1e30) is exact for m in {0,1}.
            negbig = const.tile([P, 1], F32)
            nc.vector.memset(negbig[:], NEG)
            cbt_p = tc.alloc_tile_pool(name="cbtmp", bufs=3)
            for kc in range(8):
                m_t = cbt_p.tile([P, L], U8, tag="m", name=f"m{kc}")
                nc.sync.dma_start(m_t[:], mask_d.ap()[kc * P:(kc + 1) * P, :])
                mn_t = cbt_p.tile([P, L], BF16, tag="mn", name=f"mn{kc}")
                nc.scalar.activation(mn_t[:], m_t[:], AF.Identity,
                                     bias=negbig[:], scale=-NEG)
                b_t = cbt_p.tile([P, L], BF16, tag="b", name=f"b{kc}")
                nc.sync.dma_start(b_t[:], bpp_d.ap()[kc * P:(kc + 1) * P, :])
                bs_t = cbt_p.tile([P, L], BF16, tag="bs", name=f"bs{kc}")
                nc.vector.tensor_scalar(bs_t[:], b_t[:], bw_col[:, 0:1],
                                        bb_col[:, 0:1], ALU.mult, ALU.add)
                c_t = cb_p.tile([P, L], BF16, tag="cb", name=f"cb{kc}")
                nc.gpsimd.tensor_add(c_t[:], mn_t[:], bs_t[:])
                cb.append(c_t)
            cbt_p.release()

            # qT / kT: [f, s] packed two heads per 128-partition chunk
            for which, w_t, enc_t, out_list in (
                ("q", wq_t, eq, qT), ("k", wk_t, ek, kT),
            ):
                for pc in range(4):
                    o = qkT_p.tile([P, L], BF16, tag="qkT",
                                   name=f"{which}T{pc}")
                    for sh in range(2):
                        ps = ps_s.tile([P, 512], F32, tag="ps_s",
                                       name=f"ps_{which}{pc}_{sh}")
                        for dc in range(4):
                            nc.tensor.matmul(
                                ps[:],
                                w_t[dc][:, pc * P:(pc + 1) * P],
                                enc_t[dc][:, sh * 512:(sh + 1) * 512],
                                start=(dc == 0), stop=(dc == 3))
                        sl = slice(sh * 512, (sh + 1) * 512)
                        if which == "q":
                            nc.scalar.activation(o[:, sl], ps[:], AF.Identity,
                                                 bias=wqb_sc[:, pc:pc + 1],
                                                 scale=float(SCALE))
                        else:
                            nc.scalar.activation(o[:, sl], ps[:], AF.Identity,
                                                 bias=wkb_c[:, pc:pc + 1],
                                                 scale=1.0)
                    out_list.append(o)

            # v: [s, f] with ones column interleaved per head ([128, 8*65])
            for sc in range(8):
                ps = ps_s.tile([P, 512], F32, tag="ps_s", name=f"ps_v{sc}")
                for dc in range(4):
                    nc.tensor.matmul(ps[:], ek[dc][:, sc * P:(sc + 1) * P],
                                     wv_t[dc][:], start=(dc == 0),
                                     stop=(dc == 3))
                va = v_p.tile([P, H * (HD + 1)], BF16, tag="v", name=f"v{sc}")
                vg = va[:].rearrange("p (h c) -> p h c", c=HD + 1)
                nc.vector.scalar_tensor_tensor(
                    vg[:, :, 0:HD],
                    ps[:].rearrange("p (h c) -> p h c", c=HD), 1.0,
                    wvb_bc[:].rearrange("p (h c) -> p h c", c=HD),
                    ALU.bypass, ALU.add)
                nc.vector.memset(vg[:, :, HD:HD + 1], 1.0)
                v_aug.append(va)

        # ---- wo loads (late: not projection-critical) ----
        wo_t = []
        for p_ in range(4):
            t = wo_p.tile([P, D], BF16, tag="wo", name=f"wo{p_}")
            nc.sync.dma_start(t[:], wo_d.ap()[p_ * P:(p_ + 1) * P, :])
            wo_t.append(t)

        # ---- attention ----
        # Per (head, kc): CB inject (PE, identity matmul, start) -> QK
        # accumulate (PE, stop) -> exp (ACT, -> bf16) -> PV (PE, bf16).
        # PV is emitted with a lag so the in-order PE stream never waits
        # on exp; everything between exp results is pure PE work.
        ctxn_p = ctx.enter_context(tc.tile_pool(name="ctxn", bufs=4))
        ctxr_p = ctx.enter_context(tc.tile_pool(name="ctxr", bufs=2))
        den_p = ctx.enter_context(tc.tile_pool(name="den", bufs=4))
        ctxn = [None] * 4

        def emit_norm(pc, ctxr, den_sb):
            # broadcast the pair's raw denominators (rows 0 / 32) across
            # partitions via PE (rows 0..63 get den[0], 64..127 den[32]),
            # then one full-width DVE divide per half. Nothing slow sits
            # ahead of the PE broadcast, so the PE stream stays dense.
            for qh in range(2):
                sl = slice(qh * 512, (qh + 1) * 512)
                rb_ps = ps_r.tile([P, 512], F32, tag="ps_r",
                                  name=f"rb_{pc}_{qh}")
                nc.tensor.matmul(rb_ps[:], sel[:], den_sb[:, sl],
                                 start=True, stop=True)
                rcp_sb = den_p.tile([P, 512], F32, tag="rcp",
                                    name=f"rcp_{pc}_{qh}")
                nc.vector.reciprocal(rcp_sb[:], rb_ps[:])
                nc.vector.scalar_tensor_tensor(
                    ctxn[pc][:, sl], ctxr[:, sl], 1.0, rcp_sb[:],
                    ALU.bypass, ALU.mult)

        pending_norm = None
        with tc.tile_pool(name="e", bufs=8) as e_p:
            for h in range(H):
                LAG = 4 if h == 0 else 3
                o = (h % 2) * HD
                pc = h // 2
                c_ps = [ps_c.tile([HD + 1, 512], F32, tag="ps_c",
                                  name=f"c_ps_{h}_{i}")
                        for i in range(2)]
                e_ts = {}
                for kc in range(8 + LAG):
                    if kc < 8:
                        s_ps = ps_s.tile([P, L], F32, tag="ps_s",
                                         name=f"s_ps_{h}_{kc}")
                        for qh in range(2):
                            sl = slice(qh * 512, (qh + 1) * 512)
                            nc.tensor.matmul(
                                s_ps[:, sl], ident[:], cb[kc][:, sl],
                                start=True, stop=False)
                            nc.tensor.matmul(
                                s_ps[:, sl],
                                kT[pc][o:o + HD, kc * P:(kc + 1) * P],
                                qT[pc][o:o + HD, sl],
                                start=False, stop=True)
                        et = e_p.tile([P, L], BF16, tag="e",
                                      name=f"e_{h}_{kc}")
                        nc.scalar.activation(et[:], s_ps[:], AF.Exp)
                        e_ts[kc] = et
                    if kc == 1 and pending_norm is not None:
                        # previous pair's normalize, emitted one matmul
                        # group into this head so the ACT den copies are
                        # done before the PE broadcast needs them.
                        emit_norm(*pending_norm)
                        pending_norm = None
                    if kc >= LAG:
                        kp = kc - LAG
                        for qh in range(2):
                            sl = slice(qh * 512, (qh + 1) * 512)
                            nc.tensor.matmul(
                                c_ps[qh][:],
                                v_aug[kp][:, h * (HD + 1):(h + 1) * (HD + 1)],
                                e_ts[kp][:, sl],
                                start=(kp == 0), stop=(kp == 7))
                # evict raw ctx + denominator rows (ACT, partition-shifted),
                # freeing PSUM; normalize later from SBUF.
                if h % 2 == 0:
                    ctxn[pc] = ctxn_p.tile([P, L], BF16, tag="ctxn",
                                           name=f"ctxn{pc}")
                    ctxr = ctxr_p.tile([P, L], F32, tag="ctxr",
                                       name=f"ctxr{pc}")
                    den_sb = den_p.tile([33, L], mybir.dt.float32r,
                                        tag="den", name=f"den{pc}")
                    # rows 1..31 are read (x0) by the broadcast matmul
                    nc.vector.memset(den_sb[:].bitcast(F32), 1.0)
                    ctxr_hold = (ctxr, den_sb)
                else:
                    ctxr, den_sb = ctxr_hold
                dr = (h % 2) * 32
                for qh in range(2):
                    sl = slice(qh * 512, (qh + 1) * 512)
                    nc.scalar.copy(ctxr[o:o + HD, sl], c_ps[qh][0:HD, :])
                    nc.scalar.copy(den_sb[dr:dr + 1, sl],
                                   c_ps[qh][HD:HD + 1, :])
                if h % 2 == 1:
                    pending_norm = (pc, ctxr, den_sb)
        if pending_norm is not None:
            emit_norm(*pending_norm)
            pending_norm = None

        # ---- output projection ----
        # p-major emission: all pair-0 matmuls first, so the PE only waits
        # on the last pair's normalize chain for the final 8 matmuls.
        ps_r.release()
        ps_c.release()
        ps_s.release()
        ps_o = tc.alloc_tile_pool(name="ps_o", bufs=8, space="PSUM")
        with tc.tile_pool(name="outp", bufs=3) as out_p:
            o_ps = [ps_o.tile([P, D], F32, tag="ps_o", name=f"o_ps{st}")
                    for st in range(8)]
            for p_ in range(4):
                for st in range(8):
                    nc.tensor.matmul(o_ps[st][:],
                                     ctxn[p_][:, st * P:(st + 1) * P],
                                     wo_t[p_][:],
                                     start=(p_ == 0), stop=(p_ == 3))
            for st in range(8):
                o_t = out_p.tile([P, D], F32, tag="out", name=f"out{st}")
                nc.vector.scalar_tensor_tensor(
                    o_t[:], o_ps[st][:], 1.0, wob_bc[:], ALU.bypass, ALU.add)
                nc.sync.dma_start(out_d.ap()[st * P:(st + 1) * P, :], o_t[:])
        ps_o.release()

    nc.compile()
    return nc


def to_bf16(x):
    return np.asarray(x, np.float32).astype(ml_dtypes.bfloat16)


def shard_inputs(u_enc, e_enc, logit_bpp, ue_mask, eu_mask,
                 wq_k, wq_b, wk_k, wk_b, wv_k, wv_b, wo_k, wo_b,
                 bpp_w, bpp_b):
    """Build the 8 per-core input maps (layout + bf16 rounding only)."""
    u_enc = np.asarray(u_enc, np.float32)
    e_enc = np.asarray(e_enc, np.float32)
    bpp = np.asarray(logit_bpp, np.float32)
    ue_m = np.asarray(ue_mask).astype(np.uint8)
    eu_m = np.asarray(eu_mask).astype(np.uint8)
    com = dict(
        wq=to_bf16(np.asarray(wq_k, np.float32).reshape(D, FH)),
        wk=to_bf16(np.asarray(wk_k, np.float32).reshape(D, FH)),
        wv=to_bf16(np.asarray(wv_k, np.float32).reshape(D, FH)),
        wo=to_bf16(np.asarray(wo_k, np.float32).reshape(FH, D)),
        wqb=np.asarray(wq_b, np.float32).reshape(FH).copy(),
        wkb=np.asarray(wk_b, np.float32).reshape(FH).copy(),
        wvb=np.asarray(wv_b, np.float32).reshape(FH).copy(),
        wob=np.asarray(wo_b, np.float32).reshape(D).copy(),
        bppw=np.asarray(bpp_w, np.float32).reshape(1, 1).copy(),
        bppb=np.asarray(bpp_b, np.float32).reshape(1, 1).copy(),
    )
    uT = [to_bf16(u_enc[b].T) for b in range(B)]
    eT = [to_bf16(e_enc[b].T) for b in range(B)]
    bpp_bf = to_bf16(bpp)
    bppT_bf = to_bf16(np.ascontiguousarray(bpp.T))
    in_maps = []
    for i in range(N_CORES):
        d, b = divmod(i, B)
        if d == 0:      # u queries, e keys -> u_update[b]
            m = dict(encQT=uT[b], encKT=eT[b], bpp=bppT_bf,
                     mask=np.ascontiguousarray(ue_m[b, 0].T))
        else:           # e queries, u keys -> e_update[b]
            m = dict(encQT=eT[b], encKT=uT[b], bpp=bpp_bf,
                     mask=np.ascontiguousarray(eu_m[b, 0].T))
        m.update(com)
        in_maps.append(m)
    return in_maps


_NC = None


def kernel(**inputs):
    global _NC
    if _NC is None:
        _NC = build_module()
    in_maps = shard_inputs(**inputs)
    res = bass_utils.run_bass_kernel_spmd(
        _NC, in_maps, core_ids=list(range(N_CORES)))
    u_update = np.stack([res.results[b]["out"] for b in range(B)])
    e_update = np.stack([res.results[B + b]["out"] for b in range(B)])
    return u_update, e_update


if __name__ == "__main__":
    # single-core CoreSim check of one (direction, batch) unit
    from concourse.bass_interp import CoreSim

    rng = np.random.default_rng(0)
    u = rng.standard_normal((B, L, D)).astype(np.float32)
    e = rng.standard_normal((B, L, D)).astype(np.float32)
    bpp = rng.standard_normal((L, L)).astype(np.float32)
    uem = (rng.random((B, 1, L, L)) < 0.9)
    eum = (rng.random((B, 1, L, L)) < 0.9)
    w = 1.0 / np.sqrt(D)
    wq = (rng.standard_normal((D, H, HD)) * w).astype(np.float32)
    wk = (rng.standard_normal((D, H, HD)) * w).astype(np.float32)
    wv = (rng.standard_normal((D, H, HD)) * w).astype(np.float32)
    wo = (rng.standard_normal((H, HD, D)) / np.sqrt(FH)).astype(np.float32)
    zq = (rng.standard_normal((H, HD)) * 0.1).astype(np.float32)
    zo = (rng.standard_normal((D,)) * 0.1).astype(np.float32)

    nc = build_module()
    in_maps = shard_inputs(u, e, bpp, uem, eum, wq, zq, wk, zq, wv, zq,
                           wo, zo, np.float32(1.3), np.float32(-0.2))

    core = 0
    sim = CoreSim(nc, trace=False)
    for k, vv in in_maps[core].items():
        sim.tensor(k)[:] = vv
    sim.simulate(check_with_hw=False)
    got = np.array(sim.tensor("out"))

    def ref_unit(encQ, encK, bias_qk, mask_qk):
        q = SCALE * (encQ @ wq.reshape(D, FH) + zq.reshape(FH))
        kk = encK @ wk.reshape(D, FH) + zq.reshape(FH)
        vv = encK @ wv.reshape(D, FH) + zq.reshape(FH)
        accum = np.zeros((L, D), np.float64)
        for h in range(H):
            qi = q[:, h * HD:(h + 1) * HD]
            ki = kk[:, h * HD:(h + 1) * HD]
            vi = vv[:, h * HD:(h + 1) * HD]
            s = qi @ ki.T + bias_qk
            s = np.where(mask_qk, s, -np.inf)
            s = s - s.max(-1, keepdims=True)
            p_ = np.exp(s)
            p_ /= p_.sum(-1, keepdims=True)
            accum += (p_ @ vi) @ wo[h]
        return (accum + zo).astype(np.float32)

    bq = 1.3 * bpp + -0.2
    exp_out = ref_unit(u[0], e[0], bq, uem[0, 0])
    err = np.abs(got - exp_out).max() / np.abs(exp_out).max()
    print("unit relerr vs numpy:", err)
